# revision 42
# baseline (speedup 1.0000x reference)
"""Self-contained Trainium2 Bass kernel for 16-head cross-attention MHA.

Problem: B=2, SQ=SK=2048, D=1024, H=16, key_size=64 (fp32 in/out).

Sharding (8 cores): data-parallel over batch (2) x tensor-parallel over
head groups (4 heads per core). Each core computes its 4 heads'
Q/K/V projections (column slices of wq/wk/wv), attention, and a partial
output projection (row slice of wo). Host sums the 4 partial outputs per
batch and adds the (bv @ wo + bo) correction (probs sum to 1, so bv
contributes exactly bv @ wo; bk cancels in softmax).

v2 layout (ScalarE exp is the pacer at ~1.34us per [128,1024] tile):
  - Input DMA fanned across 4 issue engines (sync: xeT halves,
    vector: xdT halves, scalar: weights) so the K0/Q0 prefix is
    DMA-bound only, not issue-serialized. xdT is resident.
  - Prefix computes the FULL K-nt0 and Q-nt0 projections (heads 0,1)
    so phase 0's dribble budget holds V + K-nt1 + Q-nt1 exactly
    (one chain per score slot).
  - Score matmuls per key-tile are emitted interleaved across the two
    heads (rows 0:64 / 64:128 -> PE 64-row tile pairs run concurrently),
    so 4 N=512 matmuls span ~2x512 cycles.
  - ctx for phase i is PHASE-SHIFTED into phase i+1 (4 PSUM chains, one
    per m-quarter; V' carries a ones column so row 64 accumulates the
    softmax denominator Z). Last phase runs predecessor ctx at 2x in its
    front half, its own ctx at 2x in the back half.
  - Tail: out-projection from ctxT against wo rows; PSUM evictions
    alternate ScalarE/VectorE; per-m-tile DMA out.
"""

import os
import sys

for _p in ("/opt/trn_rl_repo", "/root/.axon_site/_ro/trn_rl_repo"):
    if os.path.isdir(_p) and _p not in sys.path:
        sys.path.insert(0, _p)

import numpy as np
import ml_dtypes

BF16 = ml_dtypes.bfloat16

B = 2
S = 2048          # SQ == SK
D = 1024
H = 16
KEY = 64
HPC = 4           # heads per core
NPC = HPC * KEY   # 256 per-core slice of D
KT = D // 128     # 8 contraction tiles for projections
NT = NPC // 128   # 2 head-pair tiles
MC = S // 512     # 4 m-chunks of 512
JT = S // 128     # 16 key tiles

_NC = None
LAST_RESULTS = None  # BassKernelResults of the most recent run (for test.py)


def _build_nc():
    import concourse.tile as tile
    from concourse import bacc, mybir

    FP32 = mybir.dt.float32
    BF = mybir.dt.bfloat16
    AF = mybir.ActivationFunctionType

    nc = bacc.Bacc("TRN2", target_bir_lowering=False, debug=False, num_devices=8)

    xdT = nc.dram_tensor("xdT", [D, S], BF, kind="ExternalInput").ap()
    xeT = nc.dram_tensor("xeT", [D, S], BF, kind="ExternalInput").ap()
    wq_d = nc.dram_tensor("wq", [D, NPC], BF, kind="ExternalInput").ap()
    wk_d = nc.dram_tensor("wk", [D, NPC], BF, kind="ExternalInput").ap()
    wv_d = nc.dram_tensor("wv", [D, NPC], BF, kind="ExternalInput").ap()
    wo_d = nc.dram_tensor("wo", [NPC, D], BF, kind="ExternalInput").ap()
    bq_d = nc.dram_tensor("bq", [NT, 128, 1], FP32, kind="ExternalInput").ap()
    # bf16 output halves the tail DMA volume; the host upcasts and the
    # 4-partial sum keeps the rounding well inside the error budget.
    o_d = nc.dram_tensor("o", [S, D], BF, kind="ExternalOutput").ap()

    with tile.TileContext(nc) as tc:
        with (
            tc.tile_pool(name="consts", bufs=1) as consts,
            tc.tile_pool(name="acts", bufs=1) as acts,
            tc.tile_pool(name="zp", bufs=2) as zp,
            tc.tile_pool(name="up", bufs=4) as up,
            tc.tile_pool(name="zbp", bufs=4) as zbp,
            tc.tile_pool(name="osb", bufs=3) as osb,
        ):
            # ---- resident weights (issued on ScalarE HWDGE; wk/wq first:
            # they gate the prefix) ----
            wk_sb = consts.tile([128, KT, NPC], BF, tag="wk")
            nc.scalar.dma_start(wk_sb[:], wk_d.rearrange("(t p) n -> p t n", p=128))
            wq_sb = consts.tile([128, KT, NPC], BF, tag="wq")
            bq_sb = consts.tile([128, NT, 1], FP32, tag="bq")
            nc.scalar.dma_start(bq_sb[:], bq_d.rearrange("t p o -> p t o"))
            wv_sb = consts.tile([128, KT, NPC], BF, tag="wv")
            nc.gpsimd.dma_start(wv_sb[:], wv_d.rearrange("(t p) n -> p t n", p=128))
            # wo is not needed until the out-projection tail; its DMA is
            # issued during phase 0 (below) so it doesn't compete with the
            # prefix xeT/xdT stream for HBM bandwidth.
            wo_sb = consts.tile([128, NT, D], BF, tag="wo")

            # ---- activations kept resident ----
            QT_sb = acts.tile([128, NT, S], BF, tag="QT")    # [head_dim, m]
            KT_sb = acts.tile([128, NT, S], BF, tag="KT")    # [head_dim, j]
            v_sb = acts.tile([128, JT, HPC, KEY + 1], BF, tag="v")  # V' + ones col
            ctxT_sb = acts.tile([128, NT, S], BF, tag="ctxT")
            xeT_sb = acts.tile([128, KT, S], BF, tag="xeT")

            nc.vector.memset(v_sb[:, :, :, KEY:KEY + 1], 1.0)

            # ---- encoder input stream, split per (kt, m-half) on sync;
            # m-half 0 pieces land first so the prefix can start early.
            # (xeT h1 is emitted AFTER the prefix's xdT prefetch below.)
            xeT_r = xeT.rearrange("(t p) m -> p t m", p=128)
            xdT_r = xdT.rearrange("(t p) m -> p t m", p=128)
            for kt in range(KT):
                nc.sync.dma_start(xeT_sb[:, kt, 0:1024], xeT_r[:, kt, 0:1024])
            # wq rides sync behind xeT h0: it lands before the scalar-queue
            # q0a stream finishes, so the Q prefix is q0a-gated only.
            nc.sync.dma_start(wq_sb[:], wq_d.rearrange("(t p) n -> p t n", p=128))

            # ================= single PSUM pool =================
            # "ss": 2x[128,1024] (4 banks) scores / out-proj
            # "cc": 4x[128,512] (4 banks) proj chains, V chains, ctx chains
            with (
                tc.tile_pool(name="expp", bufs=34) as expp,
                tc.tile_pool(name="xdp", bufs=12) as xdp,
                tc.tile_pool(name="ps", bufs=2, space="PSUM") as ps,
                tc.tile_pool(name="cp", bufs=4, space="PSUM") as cp,
            ):
                def xd_piece(kt, mh, eng, name):
                    """Fetch a [128,1024] (kt, m-half) piece of xdT."""
                    t = xdp.tile([128, 1024], BF, tag="xd", name=name)
                    eng.dma_start(t[:], xdT_r[:, kt, mh * 1024:(mh + 1) * 1024])
                    return t

                def proj_chain(w_sb, nt, x_sb, mcs, dst, bias, name):
                    """Emit projection chains for m-chunks `mcs` (cc pool).

                    x_sb: resident [128, KT, S] AP, or a callable
                    (kt, mc) -> [128, 512] AP for streamed inputs.
                    """
                    chains = [cp.tile([128, 512], FP32, tag="cc",
                                      name=f"{name}_{mc}") for mc in mcs]
                    for kt in range(KT):
                        for i, mc in enumerate(mcs):
                            x_ap = (x_sb(kt, mc) if callable(x_sb)
                                    else x_sb[:, kt, mc * 512:(mc + 1) * 512])
                            nc.tensor.matmul(
                                chains[i][:],
                                w_sb[:, kt, nt * 128:(nt + 1) * 128],
                                x_ap,
                                start=(kt == 0), stop=(kt == KT - 1),
                            )
                    for i, mc in enumerate(mcs):
                        out_ap = dst[:, nt, mc * 512:(mc + 1) * 512]
                        if bias is not None:
                            nc.vector.tensor_scalar_add(out_ap, chains[i][:],
                                                        bias[:, nt, :])
                        else:
                            nc.vector.tensor_copy(out_ap, chains[i][:])

                def mk_pieces(mh, eng, nm, kts=range(KT)):
                    return {kt: xd_piece(kt, mh, eng, f"{nm}_{kt}")
                            for kt in kts}

                def piece_src(pieces):
                    return (lambda kt, mc:
                            pieces[kt][:, (mc % 2) * 512:(mc % 2 + 1) * 512])

                # ---- prefix: m-half 0 of K-nt0 / Q-nt0 only (heads 0,1).
                # Eagerly prefetch the 8 xdT h0 pieces (first-ever pool
                # allocs -> no WAR) on gpsimd, in parallel with xeT on sync.
                # Warm the exp table while DMA streams in.
                q1p = {}
                q0a_pieces = {}
                for kt in range(KT):
                    q0a_pieces[kt] = xd_piece(kt, 0, nc.scalar, f"q0ap_{kt}")
                # xeT m-half 1 (needed from phase-0 jt4) rides the scalar
                # hw queue BEHIND the prefix-critical q0a stream: same-queue
                # ordering defers it without a late issue.
                for kt in range(KT):
                    nc.scalar.dma_start(xeT_sb[:, kt, 1024:2048],
                                        xeT_r[:, kt, 1024:2048])
                scr = xdp.tile([128, 1], FP32, tag="xd", name="scratch_exp")
                nc.scalar.activation(scr[:], bq_sb[:, 0, :], AF.Exp,
                                     scale=0.125)

                # HAM warm-up: dummy matmuls on never-written SBUF keep the
                # PE busy (and its clock un-throttled) while the first xeT
                # and xdT pieces stream in.
                wz = acts.tile([64, 512], BF, tag="wz")
                nc.vector.memset(wz[:], 0.0)
                warm = ps.tile([128, 1024], FP32, tag="ss", name="warm")
                for i in range(18):
                    nc.tensor.matmul(
                        warm[:, (i % 2) * 512:(i % 2 + 1) * 512],
                        wz[0:64, 0:128], wz[0:64, :],
                        start=True, stop=True,
                    )

                # K-nt0 chains first (xeT lands first -> dense PE stream
                # that warms the HAM clock), then Q-nt0 as xdT pieces land.
                # Interleaving K/Q per-kt stalls the PE at every kt on the
                # later-arriving xdT piece. The nt1 m-half-0 chains (K and
                # Q) also run here: their inputs (resident xeT h0 and the
                # SAME q0a pieces) are already on chip, and the prefix PE is
                # otherwise DMA-idle — this thins the phase-0 dribble load.
                pre = [cp.tile([128, 512], FP32, tag="cc", name=f"pre_{i}")
                       for i in range(4)]
                for kt in range(KT):
                    for mc in range(2):
                        nc.tensor.matmul(
                            pre[mc][:],
                            wk_sb[:, kt, 0:128],
                            xeT_sb[:, kt, mc * 512:(mc + 1) * 512],
                            start=(kt == 0), stop=(kt == KT - 1),
                        )
                for kt in range(KT):
                    for mc in range(2):
                        nc.tensor.matmul(
                            pre[2 + mc][:],
                            wq_sb[:, kt, 0:128],
                            q0a_pieces[kt][:, mc * 512:(mc + 1) * 512],
                            start=(kt == 0), stop=(kt == KT - 1),
                        )
                for mc in range(2):
                    nc.vector.tensor_copy(KT_sb[:, 0, mc * 512:(mc + 1) * 512],
                                          pre[mc][:])
                    nc.vector.tensor_scalar_add(
                        QT_sb[:, 0, mc * 512:(mc + 1) * 512],
                        pre[2 + mc][:], bq_sb[:, 0, :])

                # shared Q-nt0 m-half-1 pieces (consumed at phase-0 jt6 AND
                # jt7), prefetched on sync BEHIND xeT h0. Emitted after the
                # pre-Q chains so the xdp slots they recycle (q0a's) have
                # their readers on record.
                q0cd = mk_pieces(1, nc.sync, "q0cd")
                # ---- phases: scores(si) + shifted ctx(si-1) + dribbles ----
                order = [(0, 0), (0, 1), (1, 0), (1, 1)]
                rows = [0, KEY]
                prev = None  # (hp, mh, exp_tiles)

                def emit_v_pair(jt0):
                    pv = [cp.tile([128, 512], FP32, tag="cc",
                                  name=f"pv_{jt0}_{d}") for d in range(2)]
                    for kt in range(KT):
                        for d in range(2):
                            nc.tensor.matmul(
                                pv[d][:, 0:NPC],
                                xeT_sb[:, kt, (jt0 + d) * 128:(jt0 + d + 1) * 128],
                                wv_sb[:, kt, :],
                                start=(kt == 0), stop=(kt == KT - 1),
                            )
                    for d in range(2):
                        nc.vector.tensor_copy(
                            v_sb[:, jt0 + d, :, 0:KEY],
                            pv[d][:, 0:NPC].rearrange("p (h k) -> p h k", h=HPC),
                        )

                def emit_ctx_step(hp, mh, jt, exp_row, ccs):
                    # ccs is indexed q-major (q*2+hh) so the ring-release
                    # order of the cc bufs matches emit_norm's q-outer group
                    # order (po odd tiles reuse these bufs in ring order).
                    for hh in range(2):
                        h = hp * 2 + hh
                        for q in range(2):
                            nc.tensor.matmul(
                                ccs[q * 2 + hh][0:KEY + 1, :],
                                v_sb[:, jt, h, :],
                                exp_row[hh][:, q * 512:(q + 1) * 512],
                                start=(jt == 0),
                                stop=(jt == JT - 1),
                            )

                def emit_norm(hp, mh, ccs):
                    # q-outer so both heads' q0 groups land first: the first
                    # half of this m-range's out-proj tiles unblocks after
                    # two groups instead of three. The u staging copy is
                    # what releases the cc PSUM bank early — the rest of the
                    # chain (bcast/recip/mul) runs off-critical from SBUF.
                    m0 = mh * 1024
                    for q in range(2):
                        for hh in range(2):
                            row = rows[hh]
                            c = ccs[q * 2 + hh]
                            u = up.tile([KEY + 1, 512], FP32, tag="u")
                            nc.vector.tensor_copy(u[:], c[0:KEY + 1, :])
                            zraw = zp.tile([1, 512], FP32, tag="z")
                            nc.vector.tensor_copy(zraw[:], u[KEY:KEY + 1, :])
                            zb = zbp.tile([KEY, 512], FP32, tag="zb")
                            nc.gpsimd.partition_broadcast(zb[:], zraw[:])
                            zbr = zbp.tile([KEY, 512], FP32, tag="zbr")
                            nc.vector.reciprocal_approx_fast(zbr[:], zb[:])
                            nc.vector.tensor_mul(
                                ctxT_sb[row:row + KEY, hp, m0 + q * 512:m0 + (q + 1) * 512],
                                u[0:KEY, :],
                                zbr[:],
                            )

                # eviction engine per out-proj tile, balancing the tail:
                # ScalarE (free once exp drains) takes most; VectorE takes
                # po(2) plus six odd tiles around its norm recip/mul work.
                PO_EV = {1: "v", 3: "v", 5: "v", 7: "v"}

                def emit_po(mt):
                    """Out-projection for one m-tile: 4 matmuls into a ps
                    ([128,1024]) or cc-pair PSUM slot, eviction on the
                    assigned engine, bf16 DMA out on sync/gpsimd."""
                    ot = osb.tile([128, D], BF, tag="ot")
                    if mt % 2 == 0:
                        po = ps.tile([128, 1024], FP32, tag="ss",
                                     name=f"po_{mt}")
                        halves = [po[:, 0:512], po[:, 512:1024]]
                        whole = po[:]
                    else:
                        po2 = [cp.tile([128, 512], FP32, tag="cc",
                                       name=f"po_{mt}_{e}") for e in range(2)]
                        halves = [po2[0][:], po2[1][:]]
                        whole = None
                    for dt in range(NT):
                        for ec in range(2):
                            nc.tensor.matmul(
                                halves[ec],
                                ctxT_sb[:, dt, mt * 128:(mt + 1) * 128],
                                wo_sb[:, dt, ec * 512:(ec + 1) * 512],
                                start=(dt == 0),
                                stop=(dt == NT - 1),
                            )
                    if PO_EV.get(mt) == "v":
                        if whole is not None:
                            nc.vector.tensor_copy(ot[:], whole)
                        else:
                            nc.vector.tensor_copy(ot[:, 0:512], halves[0])
                            nc.vector.tensor_copy(ot[:, 512:1024], halves[1])
                    else:
                        if whole is not None:
                            nc.scalar.copy(ot[:], whole)
                        else:
                            nc.scalar.copy(ot[:, 0:512], halves[0])
                            nc.scalar.copy(ot[:, 512:1024], halves[1])
                    eng = (nc.sync, nc.gpsimd)[mt % 2]
                    eng.dma_start(o_d[mt * 128:(mt + 1) * 128, :], ot[:])

                for si, (hp, mh) in enumerate(order):
                    m0 = mh * 1024
                    last = si == len(order) - 1
                    cur_cc = None
                    prev_cc = None
                    if prev is not None:
                        prev_cc = [cp.tile([128, 512], FP32, tag="cc",
                                           name=f"cc_{si}_{i}")
                                   for i in range(4)]
                    cur_exps = []
                    for jt in range(JT):
                        # ctx of the shifted (previous) phase is emitted
                        # BEFORE this jt's scores: its operands are always
                        # ready, while scores can WAR-stall on exp freeing
                        # their ss bank — in-order PE queue, so ready work
                        # must come first.
                        if prev is not None and not last:
                            emit_ctx_step(prev[0], prev[1], jt, prev[2][jt],
                                          prev_cc)
                        if last:
                            if jt < 8:
                                # front half: prev phase's ctx at 2x rate
                                for j2 in (jt * 2, jt * 2 + 1):
                                    emit_ctx_step(prev[0], prev[1], j2,
                                                  prev[2][j2], prev_cc)
                                if jt == 7:
                                    emit_norm(prev[0], prev[1], prev_cc)
                            elif jt < 15:
                                # back half: own ctx at 2x rate, consuming
                                # exps from earlier jts of this phase
                                if jt == 8:
                                    cur_cc = [cp.tile([128, 512], FP32,
                                                      tag="cc",
                                                      name=f"cc_last_{i}")
                                              for i in range(4)]
                                for j2 in ((jt - 8) * 2, (jt - 8) * 2 + 1):
                                    emit_ctx_step(hp, mh, j2, cur_exps[j2],
                                                  cur_cc)

                        # scores h-major: each head's exp is issued right
                        # after that head's two matmuls, so ScalarE starts
                        # ~2 matmuls earlier and the ss bank frees sooner
                        # (the 64-row tile pairing gives no extra column
                        # throughput, so emission order is free).
                        ets = []
                        for hh in range(2):
                            row = rows[hh]
                            ss = ps.tile([128, 1024], FP32, tag="ss",
                                         name=f"ss_{si}_{jt}_{hh}")
                            for q in range(2):
                                nc.tensor.matmul(
                                    ss[:, q * 512:(q + 1) * 512],
                                    KT_sb[row:row + KEY, hp, jt * 128:(jt + 1) * 128],
                                    QT_sb[row:row + KEY, hp, m0 + q * 512:m0 + (q + 1) * 512],
                                    start=True, stop=True,
                                )
                            et = expp.tile([128, 1024], BF, tag="exp")
                            nc.scalar.activation(et[:], ss[:], AF.Exp, scale=0.125)
                            ets.append(et)
                        cur_exps.append(ets)

                        if last and jt == 15:
                            # serial tail: ctx j2=14 (exp ready), then the
                            # first out-proj tiles to fill the exp(15) wait,
                            # then ctx j2=15.
                            emit_ctx_step(hp, mh, 14, cur_exps[14], cur_cc)
                            emit_po(0)
                            emit_ctx_step(hp, mh, 15, cur_exps[15], cur_cc)
                            emit_po(2)

                        if si == 0:
                            # Dribble schedule ordered by DMA arrival and
                            # consumer deadline: K-nt0/Q-nt0 m-half 1 first
                            # (scores jt>=8 / phase 1), V pairs (phase 1
                            # ctx), then K-nt1 / Q-nt1 (phase 2+ scores).
                            if jt < 4:
                                emit_v_pair(jt * 2)
                            elif jt == 4:
                                proj_chain(wk_sb, 0, xeT_sb, [2], KT_sb,
                                           None, "k0c")
                            elif jt == 5:
                                proj_chain(wk_sb, 0, xeT_sb, [3], KT_sb,
                                           None, "k0d")
                            elif jt == 6:
                                proj_chain(wq_sb, 0, piece_src(q0cd), [2],
                                           QT_sb, bq_sb, "q0c")
                            elif jt == 7:
                                proj_chain(wq_sb, 0, piece_src(q0cd), [3],
                                           QT_sb, bq_sb, "q0d")
                            elif jt < 12:
                                emit_v_pair((jt - 4) * 2)
                                if jt == 8:
                                    # prefetch Q1 m-half-0 pieces on the
                                    # now-idle sync queue so the jt13 chain
                                    # is not DMA-paced (pool WAR safe with
                                    # bufs=12: predecessors' readers were
                                    # emitted at jt6/7)
                                    q1p[0] = mk_pieces(0, nc.sync, "q1h0")
                                if jt == 11:
                                    # first half of the Q1 m-half-1 pieces:
                                    # these land in slots whose previous
                                    # readers (jt6/7) are emitted
                                    q1p[1] = mk_pieces(1, nc.sync, "q1h1",
                                                       range(4))
                            elif jt in (12, 14):
                                proj_chain(wk_sb, 1, xeT_sb,
                                           [jt - 12, jt - 11], KT_sb, None,
                                           f"k1_{jt}")
                                if jt == 12:
                                    nc.gpsimd.dma_start(
                                        wo_sb[:],
                                        wo_d.rearrange("(t p) n -> p t n",
                                                       p=128))
                            else:
                                qmh = (jt - 13) // 2
                                proj_chain(
                                    wq_sb, 1, piece_src(q1p[qmh]),
                                    [qmh * 2, qmh * 2 + 1], QT_sb,
                                    bq_sb, f"q1_{jt}")
                                if jt == 13:
                                    # second half: these slots' previous
                                    # readers are the jt13 chain just
                                    # emitted above
                                    q1p[1].update(
                                        mk_pieces(1, nc.sync, "q1h1b",
                                                  range(4, KT)))
                    if last:
                        emit_norm(hp, mh, cur_cc)
                    elif prev is not None:
                        emit_norm(prev[0], prev[1], prev_cc)
                    prev = (hp, mh, cur_exps)

                # ================= output projection tail ================
                # mt 0,2 were emitted inside the last phase. Order by
                # readiness: free m-half-0 ps tiles, then odd/cc tiles as
                # the scalar u-copies release the cc banks, then m-half-1
                # tiles as the (1,1) norm muls retire (q-outer order).
                for mt in (4, 6, 1, 3, 5, 7, 8, 10, 9, 12, 14, 11, 13, 15):
                    emit_po(mt)

                # hold the HAM activity estimator at full clock while the
                # last evictions and output DMAs drain: dummy matmuls into a
                # cc slot freed by an early odd tile's eviction.
                warm2 = cp.tile([128, 512], FP32, tag="cc", name="warm2")
                for i in range(22):
                    nc.tensor.matmul(warm2[:], wz[0:64, 0:128], wz[0:64, :],
                                     start=True, stop=True)

    nc.compile()
    return nc


def _get_nc():
    global _NC
    if _NC is None:
        _NC = _build_nc()
    return _NC


def _maybe_register_ntff_hook():
    """Optional: register the axon NTFF profile hook so BASS_TRACE=1 yields
    HW exec times. No-op if unavailable (e.g. the grading environment)."""
    if "antenv.axon_hooks" in sys.modules:
        return
    try:
        import types

        if "/root/.axon_site" not in sys.path and os.path.isdir("/root/.axon_site"):
            sys.path.append("/root/.axon_site")
        from trn_agent_boot.trn_boot import _ntff_profile_via_ctypes

        hook = _ntff_profile_via_ctypes("/opt/axon/libaxon_pjrt.so")
        mod = types.ModuleType("antenv.axon_hooks")
        mod.get_axon_ntff_profile_hook = lambda: hook
        mod.set_axon_ntff_profile_hook = lambda h: None
        sys.modules["antenv.axon_hooks"] = mod
    except Exception:
        pass


def kernel(decoder_output, encoder_output, wq, bq, wk, bk, wv, bv, wo, bo):
    from concourse.bass_utils import run_bass_kernel_spmd

    global LAST_RESULTS

    decoder_output = np.asarray(decoder_output, dtype=np.float32)
    encoder_output = np.asarray(encoder_output, dtype=np.float32)
    wq = np.asarray(wq, dtype=np.float32)
    wk = np.asarray(wk, dtype=np.float32)
    wv = np.asarray(wv, dtype=np.float32)
    wo = np.asarray(wo, dtype=np.float32)
    bq = np.asarray(bq, dtype=np.float32)
    bv = np.asarray(bv, dtype=np.float32)
    bo = np.asarray(bo, dtype=np.float32)
    # bk is softmax-invariant (adds a per-query constant to every logit).

    if os.environ.get("BASS_TRACE"):
        _maybe_register_ntff_hook()

    nc = _get_nc()

    xT = {}
    for b in range(B):
        xT[("d", b)] = np.ascontiguousarray(decoder_output[b].T).astype(BF16)
        xT[("e", b)] = np.ascontiguousarray(encoder_output[b].T).astype(BF16)

    in_maps = []
    for c in range(8):
        b, hg = c // 4, c % 4
        sl = slice(hg * NPC, (hg + 1) * NPC)
        in_maps.append({
            "xdT": xT[("d", b)],
            "xeT": xT[("e", b)],
            "wq": wq[:, sl].astype(BF16),
            "wk": wk[:, sl].astype(BF16),
            "wv": wv[:, sl].astype(BF16),
            "wo": np.ascontiguousarray(wo[sl, :]).astype(BF16),
            "bq": bq[sl].reshape(NT, 128, 1),
        })

    res = run_bass_kernel_spmd(nc, in_maps, core_ids=list(range(8)))
    LAST_RESULTS = res

    correction = (bv @ wo + bo).astype(np.float32)  # probs sum to 1
    out = np.zeros((B, S, D), dtype=np.float32)
    for c in range(8):
        out[c // 4] += res.results[c]["o"].astype(np.float32)
    out += correction[None, None, :]
    return out



# revision 43
# speedup vs baseline: 1.0269x; 1.0269x over previous
"""Self-contained Trainium2 Bass kernel for 16-head cross-attention MHA.

Problem: B=2, SQ=SK=2048, D=1024, H=16, key_size=64 (fp32 in/out).

Sharding (8 cores): data-parallel over batch (2) x tensor-parallel over
head groups (4 heads per core). Each core computes its 4 heads'
Q/K/V projections (column slices of wq/wk/wv), attention, and a partial
output projection (row slice of wo). Host sums the 4 partial outputs per
batch and adds the (bv @ wo + bo) correction (probs sum to 1, so bv
contributes exactly bv @ wo; bk cancels in softmax).

v2 layout (ScalarE exp is the pacer at ~1.34us per [128,1024] tile):
  - Input DMA fanned across 4 issue engines (sync: xeT halves,
    vector: xdT halves, scalar: weights) so the K0/Q0 prefix is
    DMA-bound only, not issue-serialized. xdT is resident.
  - Prefix computes the FULL K-nt0 and Q-nt0 projections (heads 0,1)
    so phase 0's dribble budget holds V + K-nt1 + Q-nt1 exactly
    (one chain per score slot).
  - Score matmuls per key-tile are emitted interleaved across the two
    heads (rows 0:64 / 64:128 -> PE 64-row tile pairs run concurrently),
    so 4 N=512 matmuls span ~2x512 cycles.
  - ctx for phase i is PHASE-SHIFTED into phase i+1 (4 PSUM chains, one
    per m-quarter; V' carries a ones column so row 64 accumulates the
    softmax denominator Z). Last phase runs predecessor ctx at 2x in its
    front half, its own ctx at 2x in the back half.
  - Tail: out-projection from ctxT against wo rows; PSUM evictions
    alternate ScalarE/VectorE; per-m-tile DMA out.
"""

import os
import sys

for _p in ("/opt/trn_rl_repo", "/root/.axon_site/_ro/trn_rl_repo"):
    if os.path.isdir(_p) and _p not in sys.path:
        sys.path.insert(0, _p)

import numpy as np
import ml_dtypes

BF16 = ml_dtypes.bfloat16

B = 2
S = 2048          # SQ == SK
D = 1024
H = 16
KEY = 64
HPC = 4           # heads per core
NPC = HPC * KEY   # 256 per-core slice of D
KT = D // 128     # 8 contraction tiles for projections
NT = NPC // 128   # 2 head-pair tiles
MC = S // 512     # 4 m-chunks of 512
JT = S // 128     # 16 key tiles

_NC = None
LAST_RESULTS = None  # BassKernelResults of the most recent run (for test.py)


def _build_nc():
    import concourse.tile as tile
    from concourse import bacc, mybir

    FP32 = mybir.dt.float32
    BF = mybir.dt.bfloat16
    AF = mybir.ActivationFunctionType

    nc = bacc.Bacc("TRN2", target_bir_lowering=False, debug=False, num_devices=8)

    xdT = nc.dram_tensor("xdT", [D, S], BF, kind="ExternalInput").ap()
    xeT = nc.dram_tensor("xeT", [D, S], BF, kind="ExternalInput").ap()
    wq_d = nc.dram_tensor("wq", [D, NPC], BF, kind="ExternalInput").ap()
    wk_d = nc.dram_tensor("wk", [D, NPC], BF, kind="ExternalInput").ap()
    wv_d = nc.dram_tensor("wv", [D, NPC], BF, kind="ExternalInput").ap()
    wo_d = nc.dram_tensor("wo", [NPC, D], BF, kind="ExternalInput").ap()
    bq_d = nc.dram_tensor("bq", [NT, 128, 1], FP32, kind="ExternalInput").ap()
    # bf16 output halves the tail DMA volume; the host upcasts and the
    # 4-partial sum keeps the rounding well inside the error budget.
    o_d = nc.dram_tensor("o", [S, D], BF, kind="ExternalOutput").ap()

    with tile.TileContext(nc) as tc:
        with (
            tc.tile_pool(name="consts", bufs=1) as consts,
            tc.tile_pool(name="acts", bufs=1) as acts,
            tc.tile_pool(name="zp", bufs=2) as zp,
            tc.tile_pool(name="up", bufs=4) as up,
            tc.tile_pool(name="zbp", bufs=4) as zbp,
            tc.tile_pool(name="osb", bufs=3) as osb,
        ):
            # ---- resident weights (issued on ScalarE HWDGE; wk/wq first:
            # they gate the prefix) ----
            wk_sb = consts.tile([128, KT, NPC], BF, tag="wk")
            nc.scalar.dma_start(wk_sb[:], wk_d.rearrange("(t p) n -> p t n", p=128))
            wq_sb = consts.tile([128, KT, NPC], BF, tag="wq")
            bq_sb = consts.tile([128, NT, 1], FP32, tag="bq")
            nc.scalar.dma_start(bq_sb[:], bq_d.rearrange("t p o -> p t o"))
            wv_sb = consts.tile([128, KT, NPC], BF, tag="wv")
            nc.gpsimd.dma_start(wv_sb[:], wv_d.rearrange("(t p) n -> p t n", p=128))
            # wo is not needed until the out-projection tail; its DMA is
            # issued during phase 0 (below) so it doesn't compete with the
            # prefix xeT/xdT stream for HBM bandwidth.
            wo_sb = consts.tile([128, NT, D], BF, tag="wo")

            # ---- activations kept resident ----
            QT_sb = acts.tile([128, NT, S], BF, tag="QT")    # [head_dim, m]
            KT_sb = acts.tile([128, NT, S], BF, tag="KT")    # [head_dim, j]
            v_sb = acts.tile([128, JT, HPC, KEY + 1], BF, tag="v")  # V' + ones col
            ctxT_sb = acts.tile([128, NT, S], BF, tag="ctxT")
            xeT_sb = acts.tile([128, KT, S], BF, tag="xeT")

            nc.vector.memset(v_sb[:, :, :, KEY:KEY + 1], 1.0)

            # ---- encoder input stream, split per (kt, m-half) on sync;
            # m-half 0 pieces land first so the prefix can start early.
            # (xeT h1 is emitted AFTER the prefix's xdT prefetch below.)
            xeT_r = xeT.rearrange("(t p) m -> p t m", p=128)
            xdT_r = xdT.rearrange("(t p) m -> p t m", p=128)
            for kt in range(KT):
                nc.sync.dma_start(xeT_sb[:, kt, 0:1024], xeT_r[:, kt, 0:1024])
            # wq rides sync behind xeT h0: it lands before the scalar-queue
            # q0a stream finishes, so the Q prefix is q0a-gated only.
            nc.sync.dma_start(wq_sb[:], wq_d.rearrange("(t p) n -> p t n", p=128))

            # ================= single PSUM pool =================
            # "ss": 2x[128,1024] (4 banks) scores / out-proj
            # "cc": 4x[128,512] (4 banks) proj chains, V chains, ctx chains
            with (
                tc.tile_pool(name="expp", bufs=34) as expp,
                tc.tile_pool(name="xdp", bufs=12) as xdp,
                tc.tile_pool(name="ps", bufs=2, space="PSUM") as ps,
                tc.tile_pool(name="cp", bufs=4, space="PSUM") as cp,
            ):
                def xd_piece(kt, mh, eng, name):
                    """Fetch a [128,1024] (kt, m-half) piece of xdT."""
                    t = xdp.tile([128, 1024], BF, tag="xd", name=name)
                    eng.dma_start(t[:], xdT_r[:, kt, mh * 1024:(mh + 1) * 1024])
                    return t

                def proj_chain(w_sb, nt, x_sb, mcs, dst, bias, name):
                    """Emit projection chains for m-chunks `mcs` (cc pool).

                    x_sb: resident [128, KT, S] AP, or a callable
                    (kt, mc) -> [128, 512] AP for streamed inputs.
                    """
                    chains = [cp.tile([128, 512], FP32, tag="cc",
                                      name=f"{name}_{mc}") for mc in mcs]
                    for kt in range(KT):
                        for i, mc in enumerate(mcs):
                            x_ap = (x_sb(kt, mc) if callable(x_sb)
                                    else x_sb[:, kt, mc * 512:(mc + 1) * 512])
                            nc.tensor.matmul(
                                chains[i][:],
                                w_sb[:, kt, nt * 128:(nt + 1) * 128],
                                x_ap,
                                start=(kt == 0), stop=(kt == KT - 1),
                            )
                    for i, mc in enumerate(mcs):
                        out_ap = dst[:, nt, mc * 512:(mc + 1) * 512]
                        if bias is not None:
                            nc.vector.tensor_scalar_add(out_ap, chains[i][:],
                                                        bias[:, nt, :])
                        else:
                            nc.vector.tensor_copy(out_ap, chains[i][:])

                def mk_pieces(mh, eng, nm, kts=range(KT)):
                    return {kt: xd_piece(kt, mh, eng, f"{nm}_{kt}")
                            for kt in kts}

                def piece_src(pieces):
                    return (lambda kt, mc:
                            pieces[kt][:, (mc % 2) * 512:(mc % 2 + 1) * 512])

                # ---- prefix: m-half 0 of K-nt0 / Q-nt0 only (heads 0,1).
                # Eagerly prefetch the 8 xdT h0 pieces (first-ever pool
                # allocs -> no WAR) on gpsimd, in parallel with xeT on sync.
                # Warm the exp table while DMA streams in.
                q1p = {}
                q0a_pieces = {}
                for kt in range(KT):
                    q0a_pieces[kt] = xd_piece(kt, 0, nc.scalar, f"q0ap_{kt}")
                # xeT m-half 1 (needed from phase-0 jt4) rides the scalar
                # hw queue BEHIND the prefix-critical q0a stream: same-queue
                # ordering defers it without a late issue.
                for kt in range(KT):
                    nc.scalar.dma_start(xeT_sb[:, kt, 1024:2048],
                                        xeT_r[:, kt, 1024:2048])
                scr = xdp.tile([128, 1], FP32, tag="xd", name="scratch_exp")
                nc.scalar.activation(scr[:], bq_sb[:, 0, :], AF.Exp,
                                     scale=0.125)

                # HAM warm-up: dummy matmuls on never-written SBUF keep the
                # PE busy (and its clock un-throttled) while the first xeT
                # and xdT pieces stream in.
                wz = acts.tile([64, 512], BF, tag="wz")
                nc.vector.memset(wz[:], 0.0)
                warm = ps.tile([128, 1024], FP32, tag="ss", name="warm")
                for i in range(18):
                    nc.tensor.matmul(
                        warm[:, (i % 2) * 512:(i % 2 + 1) * 512],
                        wz[0:64, 0:128], wz[0:64, :],
                        start=True, stop=True,
                    )

                # K-nt0 chains first (xeT lands first -> dense PE stream
                # that warms the HAM clock), then Q-nt0 as xdT pieces land.
                # Interleaving K/Q per-kt stalls the PE at every kt on the
                # later-arriving xdT piece. The nt1 m-half-0 chains (K and
                # Q) also run here: their inputs (resident xeT h0 and the
                # SAME q0a pieces) are already on chip, and the prefix PE is
                # otherwise DMA-idle — this thins the phase-0 dribble load.
                pre = [cp.tile([128, 512], FP32, tag="cc", name=f"pre_{i}")
                       for i in range(4)]
                for kt in range(KT):
                    for mc in range(2):
                        nc.tensor.matmul(
                            pre[mc][:],
                            wk_sb[:, kt, 0:128],
                            xeT_sb[:, kt, mc * 512:(mc + 1) * 512],
                            start=(kt == 0), stop=(kt == KT - 1),
                        )
                for kt in range(KT):
                    for mc in range(2):
                        nc.tensor.matmul(
                            pre[2 + mc][:],
                            wq_sb[:, kt, 0:128],
                            q0a_pieces[kt][:, mc * 512:(mc + 1) * 512],
                            start=(kt == 0), stop=(kt == KT - 1),
                        )
                for mc in range(2):
                    nc.vector.tensor_copy(KT_sb[:, 0, mc * 512:(mc + 1) * 512],
                                          pre[mc][:])
                    nc.vector.tensor_scalar_add(
                        QT_sb[:, 0, mc * 512:(mc + 1) * 512],
                        pre[2 + mc][:], bq_sb[:, 0, :])

                # shared Q-nt0 m-half-1 pieces (consumed at phase-0 jt6 AND
                # jt7), prefetched on sync BEHIND xeT h0. Emitted after the
                # pre-Q chains so the xdp slots they recycle (q0a's) have
                # their readers on record.
                q0cd = mk_pieces(1, nc.sync, "q0cd")
                # ---- phases: scores(si) + shifted ctx(si-1) + dribbles ----
                order = [(0, 0), (0, 1), (1, 0), (1, 1)]
                rows = [0, KEY]
                prev = None  # (hp, mh, exp_tiles)

                def emit_v_pair(jt0):
                    pv = [cp.tile([128, 512], FP32, tag="cc",
                                  name=f"pv_{jt0}_{d}") for d in range(2)]
                    for kt in range(KT):
                        for d in range(2):
                            nc.tensor.matmul(
                                pv[d][:, 0:NPC],
                                xeT_sb[:, kt, (jt0 + d) * 128:(jt0 + d + 1) * 128],
                                wv_sb[:, kt, :],
                                start=(kt == 0), stop=(kt == KT - 1),
                            )
                    for d in range(2):
                        nc.vector.tensor_copy(
                            v_sb[:, jt0 + d, :, 0:KEY],
                            pv[d][:, 0:NPC].rearrange("p (h k) -> p h k", h=HPC),
                        )

                def emit_ctx_step(hp, mh, jt, exp_row, ccs):
                    # ccs is indexed q-major (q*2+hh) so the ring-release
                    # order of the cc bufs matches emit_norm's q-outer group
                    # order (po odd tiles reuse these bufs in ring order).
                    for hh in range(2):
                        h = hp * 2 + hh
                        for q in range(2):
                            nc.tensor.matmul(
                                ccs[q * 2 + hh][0:KEY + 1, :],
                                v_sb[:, jt, h, :],
                                exp_row[hh][:, q * 512:(q + 1) * 512],
                                start=(jt == 0),
                                stop=(jt == JT - 1),
                            )

                def emit_norm(hp, mh, ccs):
                    # q-outer so both heads' q0 groups land first: the first
                    # half of this m-range's out-proj tiles unblocks after
                    # two groups instead of three. The u staging copy is
                    # what releases the cc PSUM bank early — the rest of the
                    # chain (bcast/recip/mul) runs off-critical from SBUF.
                    m0 = mh * 1024
                    for q in range(2):
                        for hh in range(2):
                            row = rows[hh]
                            c = ccs[q * 2 + hh]
                            u = up.tile([KEY + 1, 512], FP32, tag="u")
                            nc.vector.tensor_copy(u[:], c[0:KEY + 1, :])
                            zraw = zp.tile([1, 512], FP32, tag="z")
                            nc.vector.tensor_copy(zraw[:], u[KEY:KEY + 1, :])
                            zb = zbp.tile([KEY, 512], FP32, tag="zb")
                            nc.gpsimd.partition_broadcast(zb[:], zraw[:])
                            zbr = zbp.tile([KEY, 512], FP32, tag="zbr")
                            nc.vector.reciprocal_approx_fast(zbr[:], zb[:])
                            nc.vector.tensor_mul(
                                ctxT_sb[row:row + KEY, hp, m0 + q * 512:m0 + (q + 1) * 512],
                                u[0:KEY, :],
                                zbr[:],
                            )

                # eviction engine per out-proj tile, balancing the tail:
                # ScalarE (free once exp drains) takes most; VectorE takes
                # po(2) plus six odd tiles around its norm recip/mul work.
                PO_EV = {1: "v", 3: "v", 5: "v", 7: "v"}

                def emit_po(mt):
                    """Out-projection for one m-tile: 4 matmuls into a ps
                    ([128,1024]) or cc-pair PSUM slot, eviction on the
                    assigned engine, bf16 DMA out on sync/gpsimd."""
                    ot = osb.tile([128, D], BF, tag="ot")
                    if mt % 2 == 0:
                        po = ps.tile([128, 1024], FP32, tag="ss",
                                     name=f"po_{mt}")
                        halves = [po[:, 0:512], po[:, 512:1024]]
                        whole = po[:]
                    else:
                        po2 = [cp.tile([128, 512], FP32, tag="cc",
                                       name=f"po_{mt}_{e}") for e in range(2)]
                        halves = [po2[0][:], po2[1][:]]
                        whole = None
                    for dt in range(NT):
                        for ec in range(2):
                            nc.tensor.matmul(
                                halves[ec],
                                ctxT_sb[:, dt, mt * 128:(mt + 1) * 128],
                                wo_sb[:, dt, ec * 512:(ec + 1) * 512],
                                start=(dt == 0),
                                stop=(dt == NT - 1),
                            )
                    if PO_EV.get(mt) == "v":
                        if whole is not None:
                            nc.vector.tensor_copy(ot[:], whole)
                        else:
                            nc.vector.tensor_copy(ot[:, 0:512], halves[0])
                            nc.vector.tensor_copy(ot[:, 512:1024], halves[1])
                    else:
                        if whole is not None:
                            nc.scalar.copy(ot[:], whole)
                        else:
                            nc.scalar.copy(ot[:, 0:512], halves[0])
                            nc.scalar.copy(ot[:, 512:1024], halves[1])
                    eng = (nc.sync, nc.gpsimd)[mt % 2]
                    eng.dma_start(o_d[mt * 128:(mt + 1) * 128, :], ot[:])

                for si, (hp, mh) in enumerate(order):
                    m0 = mh * 1024
                    last = si == len(order) - 1
                    cur_cc = None
                    prev_cc = None
                    if prev is not None:
                        prev_cc = [cp.tile([128, 512], FP32, tag="cc",
                                           name=f"cc_{si}_{i}")
                                   for i in range(4)]
                    cur_exps = []
                    for jt in range(JT):
                        # ctx of the shifted (previous) phase is emitted
                        # BEFORE this jt's scores: its operands are always
                        # ready, while scores can WAR-stall on exp freeing
                        # their ss bank — in-order PE queue, so ready work
                        # must come first.
                        if prev is not None and not last:
                            emit_ctx_step(prev[0], prev[1], jt, prev[2][jt],
                                          prev_cc)
                        if last:
                            if jt < 8:
                                # front half: prev phase's ctx at 2x rate
                                for j2 in (jt * 2, jt * 2 + 1):
                                    emit_ctx_step(prev[0], prev[1], j2,
                                                  prev[2][j2], prev_cc)
                                if jt == 7:
                                    emit_norm(prev[0], prev[1], prev_cc)
                            elif jt < 15:
                                # back half: own ctx at 2x rate, consuming
                                # exps from earlier jts of this phase
                                if jt == 8:
                                    cur_cc = [cp.tile([128, 512], FP32,
                                                      tag="cc",
                                                      name=f"cc_last_{i}")
                                              for i in range(4)]
                                for j2 in ((jt - 8) * 2, (jt - 8) * 2 + 1):
                                    emit_ctx_step(hp, mh, j2, cur_exps[j2],
                                                  cur_cc)

                        # scores h-major: each head's exp is issued right
                        # after that head's two matmuls, so ScalarE starts
                        # ~2 matmuls earlier and the ss bank frees sooner
                        # (the 64-row tile pairing gives no extra column
                        # throughput, so emission order is free).
                        ets = []
                        for hh in range(2):
                            row = rows[hh]
                            ss = ps.tile([128, 1024], FP32, tag="ss",
                                         name=f"ss_{si}_{jt}_{hh}")
                            for q in range(2):
                                nc.tensor.matmul(
                                    ss[:, q * 512:(q + 1) * 512],
                                    KT_sb[row:row + KEY, hp, jt * 128:(jt + 1) * 128],
                                    QT_sb[row:row + KEY, hp, m0 + q * 512:m0 + (q + 1) * 512],
                                    start=True, stop=True,
                                )
                            et = expp.tile([128, 1024], BF, tag="exp")
                            nc.scalar.activation(et[:], ss[:], AF.Exp, scale=0.125)
                            ets.append(et)
                        cur_exps.append(ets)

                        if last and jt == 15:
                            # serial tail: ctx j2=14 (exp ready), then the
                            # first out-proj tiles to fill the exp(15) wait,
                            # then ctx j2=15.
                            emit_ctx_step(hp, mh, 14, cur_exps[14], cur_cc)
                            emit_po(0)
                            emit_ctx_step(hp, mh, 15, cur_exps[15], cur_cc)
                            emit_po(2)

                        if si == 0:
                            # Dribble schedule ordered by DMA arrival and
                            # consumer deadline: K-nt0/Q-nt0 m-half 1 first
                            # (scores jt>=8 / phase 1), V pairs (phase 1
                            # ctx), then K-nt1 / Q-nt1 (phase 2+ scores).
                            if jt < 4:
                                emit_v_pair(jt * 2)
                            elif jt == 4:
                                proj_chain(wk_sb, 0, xeT_sb, [2], KT_sb,
                                           None, "k0c")
                            elif jt == 5:
                                proj_chain(wk_sb, 0, xeT_sb, [3], KT_sb,
                                           None, "k0d")
                            elif jt == 6:
                                proj_chain(wq_sb, 0, piece_src(q0cd), [2],
                                           QT_sb, bq_sb, "q0c")
                            elif jt == 7:
                                proj_chain(wq_sb, 0, piece_src(q0cd), [3],
                                           QT_sb, bq_sb, "q0d")
                            elif jt < 12:
                                emit_v_pair((jt - 4) * 2)
                                if jt == 8:
                                    # prefetch Q1 m-half-0 pieces on the
                                    # now-idle sync queue so the jt13 chain
                                    # is not DMA-paced (pool WAR safe with
                                    # bufs=12: predecessors' readers were
                                    # emitted at jt6/7)
                                    q1p[0] = mk_pieces(0, nc.sync, "q1h0")
                                if jt == 11:
                                    # first half of the Q1 m-half-1 pieces:
                                    # these land in slots whose previous
                                    # readers (jt6/7) are emitted
                                    q1p[1] = mk_pieces(1, nc.sync, "q1h1",
                                                       range(4))
                            elif jt in (12, 14):
                                proj_chain(wk_sb, 1, xeT_sb,
                                           [jt - 12, jt - 11], KT_sb, None,
                                           f"k1_{jt}")
                                if jt == 12:
                                    nc.gpsimd.dma_start(
                                        wo_sb[:],
                                        wo_d.rearrange("(t p) n -> p t n",
                                                       p=128))
                            else:
                                qmh = (jt - 13) // 2
                                proj_chain(
                                    wq_sb, 1, piece_src(q1p[qmh]),
                                    [qmh * 2, qmh * 2 + 1], QT_sb,
                                    bq_sb, f"q1_{jt}")
                                if jt == 13:
                                    # second half: these slots' previous
                                    # readers are the jt13 chain just
                                    # emitted above
                                    q1p[1].update(
                                        mk_pieces(1, nc.sync, "q1h1b",
                                                  range(4, KT)))
                    if last:
                        emit_norm(hp, mh, cur_cc)
                    elif prev is not None:
                        emit_norm(prev[0], prev[1], prev_cc)
                    prev = (hp, mh, cur_exps)

                # ================= output projection tail ================
                # mt 0,2 were emitted inside the last phase. Order by
                # readiness: free m-half-0 ps tiles, then odd/cc tiles as
                # the scalar u-copies release the cc banks, then m-half-1
                # tiles as the (1,1) norm muls retire (q-outer order).
                for mt in (4, 6, 1, 8, 10, 3, 12, 14, 5, 7, 9, 11, 13, 15):
                    emit_po(mt)

                # hold the HAM activity estimator at full clock while the
                # last evictions and output DMAs drain: dummy matmuls into a
                # cc slot freed by an early odd tile's eviction.
                warm2 = cp.tile([128, 512], FP32, tag="cc", name="warm2")
                for i in range(22):
                    nc.tensor.matmul(warm2[:], wz[0:64, 0:128], wz[0:64, :],
                                     start=True, stop=True)

    nc.compile()
    return nc


def _get_nc():
    global _NC
    if _NC is None:
        _NC = _build_nc()
    return _NC


def _maybe_register_ntff_hook():
    """Optional: register the axon NTFF profile hook so BASS_TRACE=1 yields
    HW exec times. No-op if unavailable (e.g. the grading environment)."""
    if "antenv.axon_hooks" in sys.modules:
        return
    try:
        import types

        if "/root/.axon_site" not in sys.path and os.path.isdir("/root/.axon_site"):
            sys.path.append("/root/.axon_site")
        from trn_agent_boot.trn_boot import _ntff_profile_via_ctypes

        hook = _ntff_profile_via_ctypes("/opt/axon/libaxon_pjrt.so")
        mod = types.ModuleType("antenv.axon_hooks")
        mod.get_axon_ntff_profile_hook = lambda: hook
        mod.set_axon_ntff_profile_hook = lambda h: None
        sys.modules["antenv.axon_hooks"] = mod
    except Exception:
        pass


def kernel(decoder_output, encoder_output, wq, bq, wk, bk, wv, bv, wo, bo):
    from concourse.bass_utils import run_bass_kernel_spmd

    global LAST_RESULTS

    decoder_output = np.asarray(decoder_output, dtype=np.float32)
    encoder_output = np.asarray(encoder_output, dtype=np.float32)
    wq = np.asarray(wq, dtype=np.float32)
    wk = np.asarray(wk, dtype=np.float32)
    wv = np.asarray(wv, dtype=np.float32)
    wo = np.asarray(wo, dtype=np.float32)
    bq = np.asarray(bq, dtype=np.float32)
    bv = np.asarray(bv, dtype=np.float32)
    bo = np.asarray(bo, dtype=np.float32)
    # bk is softmax-invariant (adds a per-query constant to every logit).

    if os.environ.get("BASS_TRACE"):
        _maybe_register_ntff_hook()

    nc = _get_nc()

    xT = {}
    for b in range(B):
        xT[("d", b)] = np.ascontiguousarray(decoder_output[b].T).astype(BF16)
        xT[("e", b)] = np.ascontiguousarray(encoder_output[b].T).astype(BF16)

    in_maps = []
    for c in range(8):
        b, hg = c // 4, c % 4
        sl = slice(hg * NPC, (hg + 1) * NPC)
        in_maps.append({
            "xdT": xT[("d", b)],
            "xeT": xT[("e", b)],
            "wq": wq[:, sl].astype(BF16),
            "wk": wk[:, sl].astype(BF16),
            "wv": wv[:, sl].astype(BF16),
            "wo": np.ascontiguousarray(wo[sl, :]).astype(BF16),
            "bq": bq[sl].reshape(NT, 128, 1),
        })

    res = run_bass_kernel_spmd(nc, in_maps, core_ids=list(range(8)))
    LAST_RESULTS = res

    correction = (bv @ wo + bo).astype(np.float32)  # probs sum to 1
    out = np.zeros((B, S, D), dtype=np.float32)
    for c in range(8):
        out[c // 4] += res.results[c]["o"].astype(np.float32)
    out += correction[None, None, :]
    return out

